# revision 47
# baseline (speedup 1.0000x reference)
"""KANLayer kernel for 8 Trainium2 NeuronCores (raw Bass, explicit semaphores).

Reference computation (B=4096, D=1024, O=1024, S=4 spline points):
    xmin/xmax = per-feature min/max of x over the batch dim      # [1, D]
    xn  = (x - xmin) / (xmax - xmin)                             # [B, D]
    c   = spline_coeffs.sum(axis=2)                              # [O, D, 4]
    out = xn^3 @ c0.T + xn^2 @ c1.T + xn @ c2.T + c3.sum(d)     # [B, O]

Sharding: tensor-parallel over the output dim O. Core r owns output columns
[128r, 128r+128). Every core loads the full x (transposed to [D, B] fp16 on
the host) and computes all per-feature stats locally - no collective at all.

This environment executes NEFF engine instructions with a large fixed
per-instruction cost (measured: PE matmul ~30-43us depending on machine
phase, DVE tensor op ~20-55us, ACT ~90us, pool ~23us, DMA issue ~10us +
transfer), and a DVE+Pool concurrency probe measured the engines'
instruction processing as FULLY serialized chip-side - so the only thing
that matters is TOTAL instruction count. The 192 matmuls (8 d-chunks x
3 powers x 8 batch quarters of 512; 512 moving rows is a hard ISA limit)
are the irreducible floor; everything else is squeezed to 30 instructions:

  SP  : 1 bulk xt DMA (1024 descriptors) + 1 output DMA + 2 waits
  Pool: 1 bulk DMA (coeffs and the c3 plane packed into one 40KB-per-
        partition transfer, 128 descriptors) + 1 wait
  DVE : 15 work ops - 1 spline-sum reduce (strided over the s axis),
        2 stats reduces (min/max over [p,8,4096] in one instruction
        each), sub + reciprocal (ALU divide is invalid ISA), 1 c3 bias
        reduce, then per batch-half: broadcast (x-mn) and *sc (stride-0
        inner dim), x^2, x^3; finally one whole-PSUM drain with the bias
        folded in - plus 7 waits
  ACT : unused (most expensive engine per instruction)
  PE  : 192 fp16 matmuls in 2 half groups + 2 s_pw waits (the s_cs/s_dr
        waits are transitively implied by s_pw via DVE program order:
        x3-h0 follows the spline sum, which follows the previous drain)

Same-engine RAW hazards on DVE are NOT checked by this runtime's pipeline
(measured: a small consumer racing a just-issued producer reads stale
data, ~2-deep lookahead), so short producer->consumer chains carry s_dv
waits; equal-size streaming ops rely on the trailing-pointer property.

All tensors fp16 on the wire and in SBUF (PSUM accumulates f32); measured
end-to-end max relative error ~6e-4 vs the f32 reference (gate: 2e-2).
DRAM rows are padded (+64 elements) so DMA descriptors do not coalesce
into the single-engine contiguous path (measured 8GB/s vs ~100GB/s for
strided 128-descriptor transfers).

n_iters > 1 builds a NEFF that runs the whole kernel N times back-to-back
(for device-time measurement by wall-clock slope; the axon tunnel's
per-call input shipping makes single-run wall time meaningless).
timing_mode keeps all big tensors internal so the tunnel ships nothing.

Output per core is out_t [128, B] fp16 (transposed); the host concatenates
the 8 shards, crops the pad, transposes back and upcasts to f32.
"""

import numpy as np

import concourse.bass as bass
import concourse.mybir as mybir
from concourse.bass_utils import run_bass_kernel_spmd

P = 128            # SBUF partitions / rows per tile
B = 4096           # batch
BP = B + 64        # padded DRAM row length (defeats descriptor coalescing)
D = 1024           # input features
O = 1024           # output features
S = 4              # spline points
KC = 4             # cubic coefficients per (o, d)
NCORES = 8
OS = O // NCORES   # output columns per core = 128
DC = D // P        # d-chunks = 8
HC = DC // 2       # chunks per half = 4
QW = 512           # matmul moving-dim width (one PSUM bank)
NQ = B // QW       # 8
CF = S * KC * DC * OS   # coeff free elements per partition = 16384
C3F = D * S             # c3 natural plane row = 4096
CFT = CF + C3F          # packed coeffs + c3 plane = 20480
CFTP = CFT + 64         # padded row

F32 = mybir.dt.float32
F16 = mybir.dt.float16
AX = mybir.AxisListType
ALU = mybir.AluOpType

_CACHE = {}


def _bcast(ap2d, lo, n, inner):
    """[P, DC] slice [lo:lo+n] -> [P, n, inner] with stride-0 inner dim."""
    return bass.AP(
        tensor=ap2d.tensor,
        offset=ap2d.offset + lo,
        ap=[[ap2d.ap[0][0], P], [1, n], [0, inner]],
    )


def _build_bass(n_iters: int = 1, timing_mode: bool = False) -> bass.Bass:
    # the only SWDGE (pool) DMA generates 128 descriptors (2KB); an 8KB
    # descriptor carveout (vs the 16KB default) leaves room for the packed
    # coefficient buffer
    nc = bass.Bass(num_devices=NCORES, dynamic_dma_scratch_size=8192)

    kind = {} if timing_mode else {"kind": "ExternalInput"}
    okind = {} if timing_mode else {"kind": "ExternalOutput"}
    xt = nc.dram_tensor("xt", [D, BP], F16, **kind)
    # [p][s, k, j, o] coeffs (32KB) ++ the k=3 plane in natural layout
    # [o, d*s] (8KB, partition = o so the bias reduce lands o-major),
    # packed so ONE DMA per iteration loads both
    cf = nc.dram_tensor("cf", [P, CFTP], F16, **kind)
    out_t = nc.dram_tensor("out_t", [OS, BP], F16, **okind)
    dummy = (
        nc.dram_tensor("tout", [P, 2], F32, kind="ExternalOutput")
        if timing_mode
        else None
    )

    from contextlib import ExitStack

    ctx = ExitStack()
    with ctx:
        sem = lambda name: ctx.enter_context(nc.semaphore(name))  # noqa: E731
        s_ld = sem("s_ld")    # +32/iter: xt AND coeff+c3 bulk loads done
        s_cs = sem("s_cs")    # +1/iter: spline sum done (c_all ready)
        s_bc = sem("s_bc")    # +1/iter: bias ready (c3 region free)
        s_xn = sem("s_xn")    # +1/iter: last raw-x read done (xt_all free)
        s_pw = sem("s_pw")    # +1/half: power tiles ready (2/iter)
        s_mm = sem("s_mm")    # +1/half: PE consumed half (2/iter)
        s_dr = sem("s_dr")    # +1/iter: PSUM drained
        s_out = sem("s_out")  # +16/iter: output store done
        s_dv = sem("s_dv")    # DVE same-engine RAW chain (+4/iter)
        s_fin = sem("s_fin")  # timing-mode init/final bookkeeping

        sb = lambda name, shape, dtype=F16: ctx.enter_context(  # noqa: E731
            nc.sbuf_tensor(name, shape, dtype)
        )
        xt_all = sb("xt_all", [P, DC, B])     # 64KB/partition, stays raw
        qxn = sb("qxn", [P, HC, B])           # 32KB: xn of current half
        qx2 = sb("qx2", [P, HC, B])           # 32KB: xn^2
        qx3 = sb("qx3", [P, HC, B])           # 32KB: (x-mn) temp, then xn^3
        cboth = sb("cboth", [P, S + 1, KC * DC * OS])  # 40KB: coeffs+c3
        craw = cboth[:, 0:S, :]
        c_all = sb("c_all", [P, KC, DC, OS])      # 8KB
        # one allocation for all per-chunk stats (allocator rounds per
        # tensor, five tiny tensors would waste ~8KB/partition)
        stats = sb("stats", [P, 5 * DC], F32)
        mn = stats[:, 0:DC]
        mx = stats[:, DC : 2 * DC]
        rng = stats[:, 2 * DC : 3 * DC]
        sc = stats[:, 3 * DC : 4 * DC]
        bias = stats[:, 4 * DC : 4 * DC + 1]

        c3_tile = cboth[:, S, :]
        # output stage: x^2 chunk-slot-0 region (PE consumed it by drain)
        stage = qx2[:, 0, :]

        psum = ctx.enter_context(nc.psum_tensor("ps", [P, B], F32))

        NI = n_iters

        with nc.Block() as block:

            @block.sync
            def _(sp):
                if timing_mode:
                    # one-time finite DRAM init (per-chunk rng = 0.5)
                    sp.wait_ge(s_fin, 2)
                    z = xt_all[:, 0, :]
                    sp.dma_start(
                        out=xt[:, :].rearrange("(n p) f -> p n f", p=P),
                        in_=bass.AP(
                            tensor=z.tensor,
                            offset=z.offset,
                            ap=[[z.ap[0][0], P], [0, D // P], [1, BP]],
                        ),
                    ).then_inc(s_fin, 16)
                    sp.wait_ge(s_fin, 18)
                    sp.dma_start(
                        out=cf[:, :],
                        in_=bass.AP(
                            tensor=z.tensor,
                            offset=z.offset,
                            ap=[[z.ap[0][0], P], [1, CFTP]],
                        ),
                    ).then_inc(s_fin, 16)
                    sp.wait_ge(s_fin, 34)
                for it in range(NI):
                    if it > 0:
                        sp.wait_ge(s_xn, it)  # last raw-x read of prev iter
                    sp.dma_start(
                        out=xt_all[:, :, :],
                        in_=xt[:, 0:B].rearrange("(j p) f -> p j f", p=P),
                    ).then_inc(s_ld, 16)
                    sp.wait_ge(s_dr, it + 1)
                    sp.dma_start(
                        out=out_t[:, 0:B], in_=stage
                    ).then_inc(s_out, 16)
                sp.wait_ge(s_out, 16 * NI)
                if dummy is not None:
                    sp.dma_start(out=dummy[:, :], in_=stats[:, 16:18]).then_inc(
                        s_fin, 16
                    )
                    sp.wait_ge(s_fin, 50)

            @block.gpsimd
            def _(pool):
                for it in range(NI):
                    if it > 0:
                        # s_bc also orders the spline sum's craw read: bias
                        # increments s_bc after the spline on the DVE stream
                        pool.wait_ge(s_bc, it)
                    pool.dma_start(
                        out=cboth[:, :, :].rearrange("p s f -> p (s f)"),
                        in_=cf[:, 0:CFT],
                    ).then_inc(s_ld, 16)


            @block.vector
            def _(dve):
                if timing_mode:
                    dve.memset(xt_all[:, :, 0 : B // 2], 0.25).then_inc(s_fin)
                    dve.memset(xt_all[:, :, B // 2 : B], 0.75).then_inc(s_fin)
                def spline(it):
                    # c_all <- sum_s craw; craw holds load #it. The single
                    # merged wait also covers the xt load that the stats
                    # reduces (same DVE stream, just after) depend on.
                    dve.wait_ge(s_ld, 32 * (it + 1))
                    with nc.allow_low_precision(reason="4-val fp16 spline"):
                        dve.tensor_reduce(
                            c_all[:, :, :, :].rearrange(
                                "p k j o -> p (k j o)"
                            ),
                            craw[:, :, :].rearrange("p s f -> p f s"),
                            axis=AX.X,
                            op=ALU.add,
                        ).then_inc(s_cs)

                def stats(n):
                    # one min and one max reduce cover all 8 chunks
                    # (divide is invalid tensor_tensor ISA, so reciprocal
                    # + multiply it stays)
                    V = 4 * n
                    dve.tensor_reduce(
                        mn, xt_all[:, :, :], axis=AX.X, op=ALU.min
                    ).then_inc(s_dv)
                    dve.tensor_reduce(
                        mx, xt_all[:, :, :], axis=AX.X, op=ALU.max
                    ).then_inc(s_dv)
                    dve.wait_ge(s_dv, V + 2)
                    dve.tensor_sub(rng, mx, mn).then_inc(s_dv)
                    dve.wait_ge(s_dv, V + 3)
                    dve.reciprocal(sc, rng).then_inc(s_dv)

                for it in range(NI):
                    # (no c_all WAR wait needed: the previous drain already
                    # waited s_mm >= 2*it on this same stream)
                    spline(it)
                    stats(it)
                    # ---- c3 bias: one reduce over the natural plane
                    # (load already awaited by the spline sum) ----
                    dve.tensor_reduce(
                        bias, c3_tile, axis=AX.X, op=ALU.add
                    ).then_inc(s_bc)
                    # ---- powers, half by half (single-buffered tiles;
                    # PE frees them in DVE's write order qx3, qxn, qx2 via
                    # its k-group order and per-group s_mm increments) ----
                    # (no s_dv wait needed: this iteration's mn/sc were
                    # produced early in the previous loop body)
                    for h in range(2):
                        lo = h * HC
                        if h == 1:
                            dve.wait_ge(s_mm, 2 * it + 1)  # tiles recycled
                            d = dve.tensor_sub(
                                qx3[:, :, :],
                                xt_all[:, lo : lo + HC, :],
                                _bcast(mn, lo, HC, B),
                            )
                            d.then_inc(s_xn)  # last raw-x read
                        else:
                            if it > 0:
                                dve.wait_ge(s_out, 16 * it)  # stage stored
                            dve.tensor_sub(
                                qx3[:, :, :],
                                xt_all[:, lo : lo + HC, :],
                                _bcast(mn, lo, HC, B),
                            )
                        dve.tensor_mul(
                            qxn[:, :, :],
                            qx3[:, :, :],
                            _bcast(sc, lo, HC, B),
                        )
                        dve.tensor_mul(qx2[:, :, :], qxn[:, :, :], qxn[:, :, :])
                        dve.tensor_mul(
                            qx3[:, :, :], qxn[:, :, :], qx2[:, :, :]
                        ).then_inc(s_pw)
                    # ---- drain: psum + bias -> fp16 stage ----
                    dve.wait_ge(s_mm, 2 * (it + 1))
                    dve.tensor_scalar_add(
                        stage, psum[:, :], bias
                    ).then_inc(s_dr)

            @block.tensor
            def _(pe):
                for it in range(NI):
                    # s_pw >= 2it+1 transitively orders everything PE needs:
                    # x3-h0(it) follows spline(it) (c_all ready) which
                    # follows drain(it-1) (PSUM free) on the DVE stream, so
                    # neither an s_cs nor an s_dr wait is needed
                    for h in range(2):
                        pe.wait_ge(s_pw, 2 * it + h + 1)
                        for sl in range(HC):
                            j = h * HC + sl
                            for k in range(3):
                                # k=0: c0*x^3, 1: c1*x^2, 2: c2*xn
                                src = [qx3, qx2, qxn][k]
                                for q in range(NQ):
                                    mm = pe.matmul(
                                        psum[:, q * QW : (q + 1) * QW],
                                        lhsT=c_all[:, k, j, :],
                                        rhs=src[:, sl, q * QW : (q + 1) * QW],
                                        start=(j == 0 and k == 0),
                                        stop=(j == DC - 1 and k == 2),
                                    )
                        mm.then_inc(s_mm)

    return nc


def get_bass(n_iters: int = 1, timing_mode: bool = False) -> bass.Bass:
    key = f"nc{n_iters}_{timing_mode}"
    if key not in _CACHE:
        _CACHE[key] = _build_bass(n_iters, timing_mode)
    return _CACHE[key]


def make_in_maps(x: np.ndarray, spline_coeffs: np.ndarray):
    """Host-side sharding/marshaling only (slicing, transposes, dtype cast)."""
    x = np.asarray(x, dtype=np.float32)
    spline_coeffs = np.asarray(spline_coeffs, dtype=np.float32)

    xt = np.zeros((D, BP), dtype=np.float16)
    xt[:, 0:B] = x.T.astype(np.float16)

    in_maps = []
    for r in range(NCORES):
        shard = spline_coeffs[r * OS : (r + 1) * OS]  # [OS, D, S, KC] f32
        # [p][s, k, j, o] where d = j*128 + p
        t = shard.transpose(1, 2, 3, 0)               # [D, S, K, OS]
        t = t.reshape(DC, P, S, KC, OS).transpose(1, 2, 3, 0, 4)
        cf_np = np.zeros((P, CFTP), dtype=np.float16)
        cf_np[:, 0:CF] = t.reshape(P, CF).astype(np.float16)
        # k=3 plane natural layout [o, d*s], packed after the coeffs
        cf_np[:, CF:CFT] = (
            shard[:, :, :, 3].reshape(OS, C3F).astype(np.float16)
        )
        in_maps.append({"xt": xt, "cf": cf_np})
    return in_maps


def assemble_output(results) -> np.ndarray:
    out = np.concatenate(
        [results[r]["out_t"][:, 0:B] for r in range(NCORES)], axis=0
    )
    return np.ascontiguousarray(out.T).astype(np.float32)  # [B, O]


def run(x: np.ndarray, spline_coeffs: np.ndarray, trace: bool = False,
        n_iters: int = 1):
    """Returns (output, BassKernelResults)."""
    nc = get_bass(n_iters)
    in_maps = make_in_maps(x, spline_coeffs)
    res = run_bass_kernel_spmd(nc, in_maps, list(range(NCORES)), trace=trace)
    return assemble_output(res.results), res


def kernel(x: np.ndarray, spline_coeffs: np.ndarray) -> np.ndarray:
    out, _ = run(x, spline_coeffs, trace=False)
    return out
